# revision 49
# baseline (speedup 1.0000x reference)
"""Trainium2 Bass kernel for nn_LocalAttn: grouped local attention (3x3 window).

Sharding: 8 cores = batch(2) x H-strips(4). Each core gets a 34-row slice
(32 output rows + 1 halo row each side) of the W-and-H zero-padded input,
so all cores run one identical SPMD program.

V3 design (channel-major, pixels on the free dim, W padded to 130, bf16):
  conv1 (PE, 2 col-tiled concurrent MMs) -> tanh+BN1 (ScalarE) ->
  conv2 merged neighbor+mask in ONE [64->104] matmul + ONE BN2 ACT
  (mask rows laid out row = 32*dx + 8*dy + g across three 32-row groups,
  neighbor in rows 96:104) -> value conv (PE) + PSUM->SBUF casts ->
  nb shifts per column-half (SBUF->SBUF DMA) -> logits add (DVE bf16 2x)
  -> exp (ScalarE) -> per-half denominator in 5 AT-chunks (PE ones-MM ->
  DVE fast recip -> ScalarE bf16 copy) -> per-tile attn normalize
  (PE bcast (32,128-mode) + DVE mul) -> apply per (tile, quad):
  9 bcast matmuls issued as 3 rounds of 3 CONCURRENT row-tiled MMs
  (tile_position=(32*dx,0), disjoint PSUM banks) -> 3 fused 3-tap muls
  vs shifted v (DVE; dy=1,2 via ScalarE bf16 pab copies so muls run 2x)
  -> 2 tap-plane pre-adds (DVE bf16 2x) -> 7 identity matmuls accumulate
  (PE, deferred one iteration so they overlap the next tile's DVE muls)
  -> bf16 out copy (ScalarE, deferred) -> DMA out.

The separate softmax phase is gone: denominators/normalize pipeline into
the apply loop per half, so PE/DVE/ScalarE stay busy end-to-end.
"""

import numpy as np
import ml_dtypes

import concourse.bass as bass
import concourse.bacc as bacc
import concourse.mybir as mybir
from concourse import tile
from concourse.bass_utils import run_bass_kernel_spmd

F32 = mybir.dt.float32
BF16 = mybir.dt.bfloat16
AF = mybir.ActivationFunctionType
ALU = mybir.AluOpType

EPS = 1e-5
G = 8          # groups
B = 2
C = 256
H = W = 128
HS = 32        # output rows per core
HI = 34        # input rows per core (with halo)
WP = 130       # padded width
NIN = HI * WP          # 4420
NOUT = HS * WP         # 4160
NPAD = NIN + 2         # v free size, data at base offset 1
CT = 442               # conv pixel tile (10 tiles over 4420)
AT = 416               # apply pixel tile (10 tiles over 4160)
NCT = NIN // CT
NAT = NOUT // AT
PSB = 512              # psum bank size in f32 elements
HALF = NOUT // 2
# x load chunk boundaries (first chunk small so conv tile 0 starts early)
XBND = [0, CT, CT + 1326, CT + 2652, NIN]

MROW = 96              # mask-row layout height (3 row-groups of 32)
CROW = 104             # conv2 output rows (96 mask-layout + 8 neighbor)

# knobs
PAB_DYS = (1, 2)       # which dy rounds get ScalarE bf16 pab copies
N_PREADD = 1           # tap-plane pairs folded on the DVE before the id-chain


def _erow(k, g):
    # e/nb/mask row for window tap k (= 3*dy+dx) and group g
    dy, dx = k // 3, k % 3
    return 32 * dx + 8 * dy + g


# bundle layouts
# wb128: [128, 32 | 32 | 128 | 128 | 128] = w1 q0, w1 q1, wv q0, wv q1, id128
WB128_W = 32 + 32 + 128 + 128 + 128
# selw: [96, 18*128 sel blocks | 8 denom ones]
SELW_W = 18 * 128 + 8
# pnw: [104, 96 pnn-bcast (rows 0:8) | 2*WP hmask (rows 96:104)]
PNW_W = MROW + 2 * WP
# par: [104, 4] = s1, c1 (rows 0:64); s2b, c2b (rows 0:104)
PB_PAR = 4

_NC_CACHE = {}
DEBUG_TAPS = False


def _ap3(t, offset, free_dims):
    """Raw AP on tile t: partition dim from t, then custom free dims."""
    base = t[:]
    pstride, pcount = base.ap[0]
    return bass.AP(base.tensor, base.offset + offset, [[pstride, pcount]] + free_dims)


def _build_nc():
    nc = bacc.Bacc("TRN2", target_bir_lowering=False, debug=False, num_devices=8)

    x_d = nc.dram_tensor("x", [2, 128, NIN], BF16, kind="ExternalInput")
    wb128_d = nc.dram_tensor("wb128", [128, WB128_W], BF16, kind="ExternalInput")
    w2b_d = nc.dram_tensor("w2b", [64, CROW], BF16, kind="ExternalInput")
    selw_d = nc.dram_tensor("selw", [MROW, SELW_W], BF16, kind="ExternalInput")
    pnw_d = nc.dram_tensor("pnw", [CROW, PNW_W], BF16, kind="ExternalInput")
    par_d = nc.dram_tensor("par", [CROW, PB_PAR], F32, kind="ExternalInput")
    out_d = nc.dram_tensor("out", [2, 128, NOUT], BF16, kind="ExternalOutput")
    dbg = {}
    if DEBUG_TAPS:
        dbg["t"] = nc.dram_tensor("dbg_t", [64, NIN], BF16, kind="ExternalOutput")
        dbg["mn"] = nc.dram_tensor("dbg_mn", [CROW, NPAD], BF16, kind="ExternalOutput")
        dbg["nb"] = nc.dram_tensor("dbg_nb", [MROW, NOUT], BF16, kind="ExternalOutput")
        dbg["e"] = nc.dram_tensor("dbg_e", [MROW, NOUT], BF16, kind="ExternalOutput")
        dbg["rb"] = nc.dram_tensor("dbg_rb", [8, NOUT], BF16, kind="ExternalOutput")
        dbg["v0"] = nc.dram_tensor("dbg_v0", [128, NPAD], BF16, kind="ExternalOutput")
        dbg["v1"] = nc.dram_tensor("dbg_v1", [128, NPAD], BF16, kind="ExternalOutput")

    import os
    with tile.TileContext(nc, linearize=bool(os.environ.get("K_LINEARIZE"))) as tc:
        with (
            tc.tile_pool(name="const", bufs=1) as cp,
            tc.tile_pool(name="big", bufs=1) as bp,
        ):
            # ---- bundled weight loads first (conv tile 0 needs them) ----
            # sync queue: wb128 then x chunk 0 (conv tile 0's critical path);
            # w2b/par issue concurrently from the scalar queue
            wb128 = cp.tile([128, WB128_W], BF16, tag="wb128", name="wb128")
            nc.sync.dma_start(wb128[:], wb128_d[:])
            xq = []
            for q in range(2):
                xt = bp.tile([128, NIN], BF16, tag=f"x_{q}", name=f"x_{q}")
                xq.append(xt)
            sl0 = slice(XBND[0], XBND[1])
            for q in range(2):
                nc.sync.dma_start(xq[q][:, sl0], x_d[q, :, sl0])
            w2b = cp.tile([64, CROW], BF16, tag="w2b", name="w2b")
            nc.scalar.dma_start(w2b[:], w2b_d[:])
            part = cp.tile([CROW, PB_PAR], F32, tag="part", name="part")
            nc.scalar.dma_start(part[:], par_d[:])
            for ch in range(1, len(XBND) - 1):
                sl = slice(XBND[ch], XBND[ch + 1])
                for q in range(2):
                    nc.sync.dma_start(xq[q][:, sl], x_d[q, :, sl])

            selw = cp.tile([MROW, SELW_W], BF16, tag="selw", name="selw")
            nc.sync.dma_start(selw[:], selw_d[:])
            pnw = cp.tile([CROW, PNW_W], BF16, tag="pnw", name="pnw")
            nc.sync.dma_start(pnw[:], pnw_d[:])

            t_sb = bp.tile([64, NIN], BF16, tag="t", name="t")
            mn_sb = bp.tile([CROW, NPAD], BF16, tag="mn", name="mn")
            nb96 = bp.tile([MROW, NOUT], BF16, tag="nb96", name="nb96")
            # the mask-row layout's pad rows only need FINITE values (all
            # weights there are zero); fill them once from x at load time
            for r0 in (24, 56, 88):
                nc.sync.dma_start(nb96[r0 : r0 + 8, :], xq[0][0:8, 0:NOUT])

            w1t = [wb128[:, 0:32], wb128[:, 32:64]]
            wvt = [wb128[:, 64:192], wb128[:, 192:320]]
            id128t = wb128[:, 320:448]
            denomw = selw[0:88, 18 * 128 : 18 * 128 + 8]
            pnnw = pnw[0:8, 0:MROW]
            hmt = pnw[MROW:CROW, MROW : MROW + 2 * WP]
            s1t = part[0:64, 0:1]
            c1t = part[0:64, 1:2]
            s2t = part[0:CROW, 2:3]
            c2t = part[0:CROW, 3:4]

            e96 = bp.tile([MROW, NOUT], BF16, tag="e96", name="e96")
            r_sb = bp.tile([8, NOUT], BF16, tag="r_sb", name="r_sb")
            v_sb = [bp.tile([128, NPAD], BF16, tag=f"v_{q}", name=f"v_{q}") for q in range(2)]
            nbr = mn_sb[MROW:CROW, :]
            m96 = mn_sb[0:MROW, :]

            # nb96 pad rows get filled with (finite) copies of real rows by
            # extra shift DMAs below; all downstream weights are zero there.

            # ---- convs (conv1 + merged conv2 + value conv), tiles 0..9 ----
            with (
                tc.tile_pool(name="pc64", bufs=2, space="PSUM") as pc64,
                tc.tile_pool(name="pcm", bufs=2, space="PSUM") as pcm,
                tc.tile_pool(name="pv", bufs=3, space="PSUM") as pvp,
            ):
                def _fixups_and_shifts(half):
                    # pad-col memsets + boundary-row mask for the given row
                    # range, then the 9 shift DMAs for that output-col half.
                    r0, rn = (0, 20) if half == 0 else (20, 14)
                    lp0 = 1 + r0 * WP
                    rp0 = 1 + WP - 1 + r0 * WP
                    nc.gpsimd.memset(
                        nbr[0:8, lp0 : lp0 + (rn - 1) * WP + 1 : WP], 0.0
                    )
                    nc.gpsimd.memset(
                        nbr[0:8, rp0 : rp0 + (rn - 1) * WP + 1 : WP], 0.0
                    )
                    if half == 0:
                        nc.vector.tensor_mul(
                            nbr[0:8, 1 : 1 + WP], nbr[0:8, 1 : 1 + WP], hmt[:, 0:WP]
                        )
                    else:
                        nc.vector.tensor_mul(
                            nbr[0:8, 1 + 33 * WP : 1 + 34 * WP],
                            nbr[0:8, 1 + 33 * WP : 1 + 34 * WP],
                            hmt[:, WP : 2 * WP],
                        )
                    c0 = half * HALF
                    # shifts: split issue between the sync and scalar HWDGE
                    # queues so half-A's chain isn't serialized on one DGE
                    nq = 0
                    for dy in range(3):
                        for dx in range(3):
                            off = 1 + WP + (dy - 1) * WP + (dx - 1) + c0
                            rr = 32 * dx + 8 * dy
                            eng = nc.scalar if (half == 0 and nq % 3 == 2) else nc.sync
                            eng.dma_start(
                                nb96[rr : rr + 8, c0 : c0 + HALF],
                                nbr[0:8, off : off + HALF],
                            )
                            nq += 1
                    # logits chunk for this half (bf16 2x)
                    nc.vector.tensor_add(
                        nb96[:, c0 : c0 + HALF],
                        m96[:, 1 + WP + c0 : 1 + WP + c0 + HALF],
                        nb96[:, c0 : c0 + HALF],
                    )

                def _conv1(it):
                    sl = slice(it * CT, (it + 1) * CT)
                    pt = pc64.tile([64, CT], F32)
                    nc.tensor.matmul(
                        pt[0:32, :], w1t[0], xq[0][:, sl],
                        start=True, stop=True, tile_position=(0, 0),
                    )
                    nc.tensor.matmul(
                        pt[32:64, :], w1t[1], xq[1][:, sl],
                        start=True, stop=True, tile_position=(0, 32),
                    )
                    return pt

                # conv1 runs one tile ahead and the value MMs are emitted
                # before conv2, so the PE never head-blocks on tanh
                pt_cur = _conv1(0)
                for it in range(NCT):
                    sl = slice(it * CT, (it + 1) * CT)
                    pt_next = _conv1(it + 1) if it + 1 < NCT else None
                    pvs = []
                    for q in range(2):
                        pv = pvp.tile([128, CT], F32)
                        nc.tensor.matmul(pv[:], wvt[q], xq[q][:, sl])
                        pvs.append(pv)
                    nc.scalar.activation(
                        t_sb[:, sl], pt_cur[:], AF.Tanh, bias=c1t, scale=s1t
                    )
                    pt_cur = pt_next
                    pm = pcm.tile([CROW, CT], F32)
                    nc.tensor.matmul(pm[:], w2b, t_sb[:, sl])
                    nc.scalar.activation(
                        mn_sb[:, 1 + it * CT : 1 + (it + 1) * CT], pm[:],
                        AF.Identity, bias=c2t, scale=s2t,
                    )
                    for q in range(2):
                        vdst = v_sb[q][:, 1 + it * CT : 1 + (it + 1) * CT]
                        nc.vector.tensor_copy(vdst, pvs[q][:])
                    if it == 5:
                        _fixups_and_shifts(0)
                    if it == 7:
                        # first exp chunks early so denom chunk 0 can start
                        for cc in range(2):
                            nc.scalar.activation(
                                e96[:, cc * AT : (cc + 1) * AT],
                                nb96[:, cc * AT : (cc + 1) * AT], AF.Exp,
                            )
                    if it == 8:
                        nc.scalar.activation(
                            e96[:, 2 * AT : HALF], nb96[:, 2 * AT : HALF], AF.Exp
                        )
                    if it == NCT - 1:
                        _fixups_and_shifts(1)
                        nc.scalar.activation(
                            e96[:, HALF:NOUT], nb96[:, HALF:NOUT], AF.Exp
                        )

            # ---- merged softmax + apply; one pool set spans both halves so
            # half B's denominators overlap half A's apply tail ----
            with (
                tc.tile_pool(name="pa", bufs=2, space="PSUM") as pa3p,
                tc.tile_pool(name="anp", bufs=2, space="PSUM") as anp,
                tc.tile_pool(name="t9p", bufs=3) as t9p,
                tc.tile_pool(name="pbcp", bufs=3) as pbcp,
                tc.tile_pool(name="outp", bufs=3) as outp,
            ):
                pending_t9 = None
                pending_out = None

                def _emit_id(pend, flush):
                    nonlocal pending_out
                    t9o, qo, slo = pend
                    av = anp.tile([128, PSB], F32, tag="anp", name="anp")
                    acc = av[:, 0:AT]
                    ks = [[0, 1, 2, 3, 4, 5, 6, 7, 8],
                          [1, 2, 3, 4, 5, 6, 7, 8],
                          [1, 2, 4, 5, 6, 7, 8]][N_PREADD]
                    for j, k in enumerate(ks):
                        nc.tensor.matmul(
                            acc, id128t, t9o[:, k * AT : (k + 1) * AT],
                            start=(j == 0), stop=(j == len(ks) - 1),
                            skip_group_check=True,
                        )
                    if flush and pending_out is not None:
                        pacc0, pq, psl = pending_out
                        ot = outp.tile([128, AT], BF16, tag="ot", name="ot")
                        nc.scalar.copy(ot[:], pacc0)
                        nc.sync.dma_start(out_d[pq, :, psl], ot[:])
                    pending_out = (acc, qo, slo)

                for half in range(2):
                    c0 = half * HALF
                    # denominator in 5 AT chunks: ones-MM -> fast recip
                    # -> bf16 copy to r_sb (uses the shared anp ring)
                    for cch in range(NAT // 2):
                        asl = slice(c0 + cch * AT, c0 + (cch + 1) * AT)
                        pst = anp.tile([128, PSB], F32, tag="anp", name="anp")
                        ps = pst[0:8, 0:AT]
                        nc.tensor.matmul(ps, denomw, e96[0:88, asl])
                        nc.vector.reciprocal_approx_fast(ps, ps)
                        nc.scalar.copy(r_sb[:, asl], ps)

                    for it5 in range(NAT // 2):
                        it = half * (NAT // 2) + it5
                        asl = slice(it * AT, (it + 1) * AT)
                        # normalize this tile's attn: pnn bcast + in-place mul
                        pv96 = anp.tile([128, PSB], F32, tag="anp", name="anp")
                        nc.tensor.matmul(pv96[0:MROW, 0:AT], pnnw, r_sb[:, asl])
                        nc.vector.tensor_mul(
                            e96[:, asl], e96[:, asl], pv96[0:MROW, 0:AT]
                        )
                        for q in range(2):
                            t9 = t9p.tile([128, 9 * AT], BF16, tag="t9", name="t9")
                            for dy in range(3):
                                pa = pa3p.tile([128, 3 * PSB], F32, tag="pa3", name="pa3")
                                for dx in range(3):
                                    k = 3 * dy + dx
                                    blk = 128 * (9 * q + k)
                                    nc.tensor.matmul(
                                        pa[:, dx * PSB : dx * PSB + AT],
                                        selw[32 * dx : 32 * dx + 32, blk : blk + 128],
                                        e96[32 * dx : 32 * dx + 32, asl],
                                        tile_position=(32 * dx, 0),
                                    )
                                out_ap = _ap3(t9, 3 * dy * AT, [[AT, 3], [1, AT]])
                                in1_ap = _ap3(
                                    v_sb[q], dy * WP + it * AT, [[1, 3], [1, AT]]
                                )
                                if dy in PAB_DYS:
                                    # ScalarE converts the bcast to bf16 SBUF
                                    # so this row's mul runs at 2x on the DVE
                                    pab = pbcp.tile([128, 3 * AT], BF16, tag="pab", name="pab")
                                    nc.scalar.copy(
                                        pab[:], _ap3(pa, 0, [[PSB, 3], [1, AT]])
                                    )
                                    in0_ap = _ap3(pab, 0, [[AT, 3], [1, AT]])
                                else:
                                    in0_ap = _ap3(pa, 0, [[PSB, 3], [1, AT]])
                                nc.vector.tensor_mul(out_ap, in0_ap, in1_ap)
                            # fold tap-plane pairs into neighbors (bf16 2x)
                            # so the PE accumulation streams fewer planes
                            if N_PREADD >= 1:
                                nc.vector.tensor_add(
                                    t9[:, 1 * AT : 2 * AT], t9[:, 1 * AT : 2 * AT],
                                    t9[:, 0 * AT : 1 * AT],
                                )
                            if N_PREADD >= 2:
                                nc.vector.tensor_add(
                                    t9[:, 4 * AT : 5 * AT], t9[:, 4 * AT : 5 * AT],
                                    t9[:, 3 * AT : 4 * AT],
                                )
                            # deferred id-chain of the PREVIOUS (it, q): PE
                            # sums its planes while the DVE works on this one
                            if pending_t9 is not None:
                                _emit_id(pending_t9, flush=True)
                            pending_t9 = (t9, q, asl)
                # drain the software pipeline after both halves
                if pending_t9 is not None:
                    _emit_id(pending_t9, flush=True)
                    pending_t9 = None
                if pending_out is not None:
                    pacc0, pq, psl = pending_out
                    ot = outp.tile([128, AT], BF16, tag="ot", name="ot")
                    nc.scalar.copy(ot[:], pacc0)
                    nc.sync.dma_start(out_d[pq, :, psl], ot[:])
                    pending_out = None

            if DEBUG_TAPS:
                nc.sync.dma_start(dbg["t"][:], t_sb[:])
                nc.sync.dma_start(dbg["mn"][:], mn_sb[:])
                nc.sync.dma_start(dbg["nb"][:], nb96[:])
                nc.sync.dma_start(dbg["e"][:], e96[:])
                nc.sync.dma_start(dbg["rb"][:], r_sb[:])
                nc.sync.dma_start(dbg["v0"][:], v_sb[0][:])
                nc.sync.dma_start(dbg["v1"][:], v_sb[1][:])

    nc.compile()
    return nc


def _host_prep(x, w1, b1, g1, be1, m1, v1, w2, b2, g2, be2, m2, v2, wv):
    f32 = np.float32
    bf16 = ml_dtypes.bfloat16

    inv1 = (g1 / np.sqrt(v1 + EPS)).astype(f32)            # [64]
    s1 = inv1
    c1 = (b1 * inv1 + be1 - m1 * inv1).astype(f32)
    inv2 = (g2 / np.sqrt(v2 + EPS)).astype(f32)            # [80]
    s2r = inv2
    c2r = (b2 * inv2 + be2 - m2 * inv2).astype(f32)

    # conv2 merged output rows: mask at _erow(k,g) <- ref ch 8+9g+k,
    # neighbor at 96+g <- ref ch g
    rowch = np.full(CROW, -1, dtype=np.int64)
    for k in range(9):
        for g in range(8):
            rowch[_erow(k, g)] = 8 + 9 * g + k
    for g in range(8):
        rowch[MROW + g] = g

    s2b = np.zeros(CROW, dtype=f32)
    c2b = np.zeros(CROW, dtype=f32)
    w2b = np.zeros((64, CROW), dtype=bf16)
    for j in range(CROW):
        ch = rowch[j]
        if ch < 0:
            continue
        gc, co = ch // 10, ch % 10
        s2b[j] = s2r[ch]
        c2b[j] = c2r[ch]
        w2b[8 * gc : 8 * gc + 8, j] = w2[gc, co, :].astype(bf16)

    par = np.zeros((CROW, PB_PAR), dtype=f32)
    par[0:64, 0] = s1
    par[0:64, 1] = c1
    par[:, 2] = s2b
    par[:, 3] = c2b

    # wb128 bundle: w1 block-diag per quad [128, 32]x2, wv block-diag
    # [128, 128]x2, id128
    wb128 = np.zeros((128, WB128_W), dtype=bf16)
    for q in range(2):
        for gh in range(4):
            g = 4 * q + gh
            wb128[32 * gh : 32 * gh + 32, 32 * q + 8 * gh : 32 * q + 8 * gh + 8] = (
                w1[g].T.astype(bf16)
            )
            wb128[32 * gh : 32 * gh + 32, 64 + 128 * q + 32 * gh : 64 + 128 * q + 32 * gh + 32] = (
                wv[g].T.astype(bf16)
            )
    wb128[:, 320:448] = np.eye(128, dtype=bf16)

    # selw bundle [96, 18*128 + 8]: block (q,k): row _erow(k,g) ->
    # cols 32gh..32gh+32 for g = 4q+gh; then denom ones cols
    selw = np.zeros((MROW, SELW_W), dtype=bf16)
    for q in range(2):
        for k in range(9):
            for gh in range(4):
                g = 4 * q + gh
                selw[_erow(k, g), 128 * (9 * q + k) + 32 * gh : 128 * (9 * q + k) + 32 * gh + 32] = 1
    for k in range(9):
        for g in range(8):
            selw[_erow(k, g), 18 * 128 + g] = 1

    # pnw bundle [104, 96 + 2*WP]: pnn bcast map (rows 0:8, row g -> all
    # _erow(k,g)), then per-shard hmask in rows 96:104
    pnw_base = np.zeros((CROW, PNW_W), dtype=bf16)
    for k in range(9):
        for g in range(8):
            pnw_base[g, _erow(k, g)] = 1
    hm_off = MROW

    # padded input: (2, 256, 130, 130), bf16
    xp = np.zeros((B, C, H + 2, W + 2), dtype=bf16)
    xp[:, :, 1:-1, 1:-1] = x.astype(bf16)

    shards = []
    for b in range(B):
        for qh in range(4):
            xs = xp[b, :, qh * HS : qh * HS + HI, :]       # [256, 34, 130]
            xs = np.ascontiguousarray(xs.reshape(2, 128, NIN))
            pnw = pnw_base.copy()
            pnw[MROW:CROW, hm_off : hm_off + 2 * WP] = 1
            if qh == 0:
                pnw[MROW:CROW, hm_off : hm_off + WP] = 0
            if qh == 3:
                pnw[MROW:CROW, hm_off + WP : hm_off + 2 * WP] = 0
            shards.append(
                {
                    "x": xs,
                    "wb128": wb128, "w2b": w2b, "selw": selw,
                    "pnw": pnw, "par": par,
                }
            )
    return shards


def kernel(**inputs):
    if "nc" not in _NC_CACHE:
        _NC_CACHE["nc"] = _build_nc()
    nc = _NC_CACHE["nc"]

    shards = _host_prep(**inputs)
    res = run_bass_kernel_spmd(nc, shards, core_ids=list(range(8)))

    out = np.zeros((B, C, H, W), dtype=np.float32)
    for i, r in enumerate(res.results):
        b, qh = divmod(i, 4)
        o = r["out"].astype(np.float32).reshape(C, HS, WP)[:, :, 1 : 1 + W]
        out[b, :, qh * HS : (qh + 1) * HS, :] = o
    return out
